# revision 17
# baseline (speedup 1.0000x reference)
"""Trainium2 Bass kernel for nn_LocalAttention (T=4096, B=32, H=256, L=512, K=32).

Sharding: data-parallel over batch B across 8 cores (BC=4 batch elements/core).

v2.1 dataflow (h on partitions, per core):
  - coeff GEMM col-tiled 4x (m=32 replicated lm at col groups), Wk as fp8
    e3m4 in 4 persistent 1MB tiles, i-outer accumulation so matmuls chase
    the DMA stream.  kernT copies split DVE/ACT, DRAM bounce 4+4 DMAs with
    multi-dim partition APs.
  - conv (K=32) row-tiled via tile_position with win+coef replicated at
    partition bases {0,32,64,96}; enc (fp8) added on the PE by identity
    matmul accumulation into the conv PSUM banks.
  - ACT tanh reads PSUM FD=1024 (2-tt packs), bias = per-partition glob.
  - score: stationary ws (m=32 replicated), tan streams, col-tiled over the
    tt quad into a dedicated 1-bank score pool (2 bufs) so the conv pipeline
    never waits on score/softmax.
  - softmax: scores consolidated into one (32,512) SBUF tile; exp is a
    single ACT call; per-b sums via selector matmuls.
  - DMA: triggers cost ~600ns each and queues are in-order, so loads are
    batched into few instructions and spread over the sync/gpsimd queues.
"""

import os
import sys

import numpy as np

if "/opt/trn_rl_repo" not in sys.path:
    sys.path.insert(0, "/opt/trn_rl_repo")

import ml_dtypes

T, B, H, L, K = 4096, 32, 256, 512, 32
NCORES = 8
BC = B // NCORES          # 4 batches per core
HCH = H // 128            # 2 h-chunks
TTILE = 512
NTT = T // TTILE          # 8 t-tiles per (b, hc)

_CACHE = {}


def _build_program():
    import concourse.bacc as bacc
    import concourse.bass as bass
    import concourse.mybir as mybir
    import concourse.tile as tile
    from contextlib import ExitStack

    dt = mybir.dt
    fp32 = dt.float32
    fp16 = dt.float16
    f8 = dt.float8e3
    ts = bass.ts

    nc = bacc.Bacc(
        "TRN2",
        target_bir_lowering=False,
        debug=False,
        enable_asserts=False,
        num_devices=NCORES,
    )

    enc = nc.dram_tensor("enc", (BC, HCH, 128, T), f8, kind="ExternalInput").ap()
    win = nc.dram_tensor("win", (128, T), fp16, kind="ExternalInput").ap()
    wkp = nc.dram_tensor("wkp", (4, 128, 16, 512), f8, kind="ExternalInput").ap()
    bkp = nc.dram_tensor("bkp", (1, K * H), f8, kind="ExternalInput").ap()
    bl8 = nc.dram_tensor("bl8", (128, 256), f8, kind="ExternalInput").ap()
    bl32 = nc.dram_tensor("bl32", (128, 4 * H + 16), fp32, kind="ExternalInput").ap()
    bgp = nc.dram_tensor("bgp", (1, H), fp32, kind="ExternalInput").ap()
    wsr = nc.dram_tensor("wsr", (128, HCH, 32), fp16, kind="ExternalInput").ap()
    mskl = nc.dram_tensor("mskl", (32, TTILE + 32), fp32, kind="ExternalInput").ap()
    att = nc.dram_tensor("att", (32, TTILE), fp32, kind="ExternalOutput").ap()

    TanhF = mybir.ActivationFunctionType.Tanh
    ExpF = mybir.ActivationFunctionType.Exp
    Add = mybir.AluOpType.add

    with tile.TileContext(nc) as tc, ExitStack() as ctx:
        # ---------- pools ----------
        small_pool = ctx.enter_context(tc.tile_pool(name="small", bufs=1))
        enc_pool = ctx.enter_context(tc.tile_pool(name="encp", bufs=1))
        win_pool = ctx.enter_context(tc.tile_pool(name="winp", bufs=1))
        tan_pool = ctx.enter_context(tc.tile_pool(name="tanp", bufs=17))
        scat_pool = ctx.enter_context(tc.tile_pool(name="scatp", bufs=2))
        conv_ps = ctx.enter_context(tc.tile_pool(name="cvps", bufs=3, space="PSUM"))
        score_ps = ctx.enter_context(tc.tile_pool(name="scps", bufs=2, space="PSUM"))
        dram_pool = ctx.enter_context(tc.tile_pool(name="dramp", bufs=1, space="DRAM"))

        # ---------- input loads ----------
        # sync queue (in-order): bl8, wk x4, enc x8  (critical path first)
        bl8_sb = small_pool.tile([128, 256], f8)
        nc.sync.dma_start(bl8_sb[:], bl8)
        lm8_v = bl8_sb[:, 0:128].rearrange("p (i b) -> p i b", i=4)   # (128,4,32)
        id_v = bl8_sb[:, 128:256]

        # head-critical smalls on gpsimd/sync
        bk_sb = small_pool.tile([1, K * H], f8)
        nc.sync.dma_start(bk_sb[:], bkp)
        mskl_sb = small_pool.tile([32, TTILE + 32], fp32)
        nc.gpsimd.dma_start(mskl_sb[:], mskl)
        wsr_sb = small_pool.tile([128, HCH, 32], fp16)
        nc.gpsimd.dma_start(wsr_sb[:], wsr)
        bg_sb = small_pool.tile([1, H], fp32)
        nc.gpsimd.dma_start(bg_sb[:], bgp)

        # wk: 8 half-chunks spread over the 3 trigger queues
        wk_tiles = []
        wk_eng = {(0, 0): nc.sync, (0, 1): nc.sync, (1, 0): nc.sync,
                  (1, 1): nc.scalar, (2, 0): nc.scalar, (2, 1): nc.scalar,
                  (3, 0): nc.gpsimd, (3, 1): nc.gpsimd}
        for i in range(4):
            wk_sb = small_pool.tile([128, 16, TTILE], f8, tag=f"wk{i}")
            for h in range(2):
                wk_eng[(i, h)].dma_start(
                    wk_sb[:, 8 * h : 8 * h + 8, :], wkp[i, :, 8 * h : 8 * h + 8, :]
                )
            wk_tiles.append(wk_sb)

        # win: single (128, T) tile, b-major k on partitions
        win_sb = win_pool.tile([128, T], fp16)
        nc.gpsimd.dma_start(win_sb[:], win)

        bl32_sb = small_pool.tile([128, 4 * H + 16], fp32)
        nc.scalar.dma_start(bl32_sb[:], bl32)
        wgt_v = bl32_sb[:, 0 : 4 * H].rearrange("p (i h) -> p i h", i=4)
        lmf_v = bl32_sb[:, 4 * H : 4 * H + 16].rearrange("p (i b) -> p i b", i=4)

        enc_tiles = []
        enc_eng = {0: nc.sync, 1: nc.scalar, 2: nc.gpsimd}
        for b in range(BC):
            row = []
            for hc in range(HCH):
                e_sb = enc_pool.tile([128, T], f8, tag=f"enc{b}_{hc}")
                enc_eng[(2 * b + hc) % 3].dma_start(e_sb[:], enc[b, hc])
                row.append(e_sb)
            enc_tiles.append(row)

        one8_sb = small_pool.tile([1, 32], f8)
        nc.vector.memset(one8_sb[:], 1.0)
        one_f = small_pool.tile([1, BC], fp32)
        nc.vector.memset(one_f[:], 1.0)

        # ---------- coeff GEMM: col-tiled x4 over jj, i-outer ----------
        # cps[32jj+b, jg*512+c] = (lm @ WkP + bk) for kh col n=(jg*4+jj)*512+c
        kernT = small_pool.tile([128, 4, TTILE], fp16)
        cpsA = conv_ps.tile([128, 1024], fp32, tag="cps", name="coeffA")
        cpsB = conv_ps.tile([128, 1024], fp32, tag="cps", name="coeffB")
        cps_slice = lambda jg: (cpsA if jg < 2 else cpsB)[:, ts(jg % 2, TTILE)]
        bk_v = bk_sb[:].rearrange("p (j n) -> p j n", j=16)
        for i in range(4):
            for jg in range(4):
                for jj in range(4):
                    nc.tensor.matmul(
                        cps_slice(jg)[32 * jj : 32 * jj + 32, :],
                        lm8_v[:, i, :],
                        wk_tiles[i][:, jg * 4 + jj, :],
                        start=(i == 0),
                        stop=False,
                        skip_group_check=True,
                        tile_position=(0, 32 * jj),
                    )
        for jg in range(4):
            for jj in range(4):
                nc.tensor.matmul(
                    cps_slice(jg)[32 * jj : 32 * jj + 32, :],
                    one8_sb[:],
                    bk_v[:, jg * 4 + jj, :],
                    start=False,
                    stop=True,
                    skip_group_check=True,
                    tile_position=(0, 32 * jj),
                )
            if jg % 2 == 0:
                nc.vector.tensor_copy(kernT[:, jg, :], cps_slice(jg))
            else:
                nc.scalar.copy(kernT[:, jg, :], cps_slice(jg))

        # bounce through DRAM: scr[b, k, hcc, h]; k = 8jg + 2jj + chi
        scr = dram_pool.tile([BC, K, HCH, 128], fp16)
        scr_v = scr[:].rearrange("b (jg jjj chi) c h -> jjj b jg chi c h", jg=4, jjj=4)
        for jj in range(4):
            src = kernT[32 * jj : 32 * jj + BC, :, :].rearrange(
                "b jg (chi c h) -> b jg chi c h", chi=2, c=HCH
            )
            nc.sync.dma_start(scr_v[jj], src)
        # gather back: partitions 32b+k, one DMA
        coefr = small_pool.tile([128, HCH, 128], fp16)
        nc.sync.dma_start(coefr[:], scr[:].rearrange("b k c h -> (b k) c h"))

        # ---------- glob: glob[h, b] = lm @ Wg.T + bg ----------
        glob_sb = small_pool.tile([128, HCH, BC], fp32)
        gps = conv_ps.tile([128, 1024], fp32, tag="cps", name="globps")
        for hc in range(HCH):
            gout = gps[:, 512 * hc : 512 * hc + BC]
            for i in range(4):
                nc.tensor.matmul(
                    gout,
                    wgt_v[:, i, ts(hc, 128)],
                    lmf_v[:, i, :],
                    start=(i == 0),
                    stop=False,
                )
            nc.tensor.matmul(gout, bg_sb[:, ts(hc, 128)], one_f[:], start=False, stop=True)
            nc.vector.tensor_copy(glob_sb[:, hc, :], gout)

        # ---------- main stream ----------
        scoreall = small_pool.tile([32, TTILE], fp32)

        def emit_score(b, q, tans):
            # score: stationary replicated ws (m=32), col-tiled over quad
            stile = score_ps.tile([128, TTILE], fp32, tag="sc")
            for hc in range(HCH):
                for i in range(4):
                    nc.tensor.matmul(
                        stile[32 * i : 32 * i + 32, :],
                        wsr_sb[:, hc, :],
                        tans[hc][:, ts(i, TTILE)],
                        start=(hc == 0),
                        stop=(hc == HCH - 1),
                        skip_group_check=True,
                        tile_position=(0, 32 * i),
                    )
            scat = scat_pool.tile([128, TTILE], fp32, tag="scat")
            nc.vector.tensor_copy(scat[:], stile[:])
            scat_v = scat[:].rearrange("(j r) c -> j r c", j=4)
            r0 = b * 8 + q * 4
            nc.gpsimd.dma_start(scoreall[r0 : r0 + 4, :], scat_v[:, 0, :])

        tan_tiles = {}
        for q in range(2):
            for hc in range(HCH):
                for half in range(2):
                    for b in range(BC):
                        if half == 0:
                            tan_sb = tan_pool.tile([128, 2048], fp16, tag="tan")
                            tan_tiles[(b, hc)] = tan_sb
                        else:
                            tan_sb = tan_tiles[(b, hc)]
                        cpt = conv_ps.tile([128, 1024], fp32, tag="cps", name="convps")
                        for j in range(2):
                            tt = q * 4 + 2 * half + j
                            nc.tensor.matmul(
                                cpt[:, ts(j, TTILE)],
                                coefr[32 * b : 32 * b + 32, hc, :],
                                win_sb[32 * b : 32 * b + 32, ts(tt, TTILE)],
                                start=True,
                                stop=False,
                                skip_group_check=True,
                                tile_position=(32 * b, 0),
                            )
                        for j in range(2):
                            tt = q * 4 + 2 * half + j
                            nc.tensor.matmul(
                                cpt[:, ts(j, TTILE)],
                                id_v,
                                enc_tiles[b][hc][:, ts(tt, TTILE)],
                                start=False,
                                stop=True,
                                skip_group_check=True,
                            )
                        nc.scalar.activation(
                            tan_sb[:, ts(half, 1024)],
                            cpt[:],
                            TanhF,
                            bias=glob_sb[:, hc, b : b + 1],
                            scale=1.0,
                        )
            if q > 0:
                for b in range(BC):
                    emit_score(b, 0, prev_tans[b])
            prev_tans = {b: [tan_tiles[(b, 0)], tan_tiles[(b, 1)]] for b in range(BC)}
        for b in range(BC):
            emit_score(b, 1, prev_tans[b])

        # ---------- softmax over T (per batch) ----------
        sc2 = small_pool.tile([32, TTILE], fp32)
        nc.vector.tensor_tensor(sc2[:], scoreall[:], mskl_sb[:, 0:TTILE], Add)
        esb = small_pool.tile([32, TTILE], fp32)
        rsum = small_pool.tile([32, 1], fp32)
        nc.scalar.activation(esb[:], sc2[:], ExpF, bias=0.0, scale=1.0,
                             accum_out=rsum[:])
        spt = score_ps.tile([128, TTILE], fp32, tag="sc", name="smps")
        nc.tensor.matmul(
            spt[0:32, 0:1], mskl_sb[:, TTILE : TTILE + 32], rsum[:], start=True, stop=True
        )
        rec = small_pool.tile([32, 1], fp32)
        nc.vector.reciprocal(rec[:], spt[0:32, 0:1])
        attall = small_pool.tile([32, TTILE], fp32)
        nc.vector.tensor_scalar_mul(attall[:], esb[:], rec[:])
        nc.sync.dma_start(att, attall[:])

    nc.compile()
    return nc


def _get_program():
    if "nc" not in _CACHE:
        _CACHE["nc"] = _build_program()
    return _CACHE["nc"]


def _prep_inputs(encoded_contribution, mask, lm_state, prev_att_weights,
                 Wk, bk, Wg, bg, Ws, bs):
    """Host-side shard + layout prep. Returns list of per-core input dicts."""
    f32 = np.float32
    f8 = ml_dtypes.float8_e3m4

    enc = np.asarray(encoded_contribution, dtype=f32)
    mask = np.asarray(mask, dtype=f32)
    lm = np.asarray(lm_state, dtype=f32)
    prev = np.asarray(prev_att_weights, dtype=f32)
    Wk = np.asarray(Wk, dtype=f32)
    bk = np.asarray(bk, dtype=f32)
    Wg = np.asarray(Wg, dtype=f32)
    bg = np.asarray(bg, dtype=f32)
    Ws = np.asarray(Ws, dtype=f32)
    bs = np.asarray(bs, dtype=f32)

    # enc: (T, B, H) -> (B, H, T) -> (NCORES, BC, HCH, 128, T) fp8 e3m4
    enc_t = np.ascontiguousarray(enc.transpose(1, 2, 0)).astype(f8).reshape(
        NCORES, BC, HCH, 128, T
    )

    # toeplitz windows: win[b, k, t] = prev_pad[b, k + t]
    prev_pad = np.zeros((B, T + K - 1), dtype=f32)
    prev_pad[:, K - 1 :] = prev.T
    win_full = np.lib.stride_tricks.sliding_window_view(prev_pad, T, axis=1)
    win_full = win_full.astype(np.float16).reshape(NCORES, 128, T)

    # WkP[l, k*256+h] = Wk[h*32+k, l]; dram layout (4 i, 128 p, 16 j, 512 n)
    wkp = (
        Wk.reshape(H, K, L)
        .transpose(2, 1, 0)          # (L, K, H)
        .reshape(L, K * H)
        .astype(f8)
        .reshape(4, 128, 16, 512)
    )
    wkp = np.ascontiguousarray(wkp)

    # bk permuted to [k*256+h]
    bkp = np.ascontiguousarray(bk.reshape(H, K).T.reshape(1, K * H)).astype(f8)

    # lmT chunks: (128, 4, B); per core slice then tile to 32 cols
    lmT = np.ascontiguousarray(lm.T.reshape(4, 128, B).transpose(1, 0, 2))

    # WgT chunks: (128, 4, H) flattened into bl32
    wgt = np.ascontiguousarray(Wg.T.reshape(4, 128, H).transpose(1, 0, 2))
    bgp = np.ascontiguousarray(bg.reshape(1, H))

    # ws replicated: (128, HCH, 32)
    wsv = Ws[0].reshape(HCH, 128).T.astype(np.float16)      # (128, HCH)
    wsr = np.ascontiguousarray(np.repeat(wsv[:, :, None], 32, axis=2))

    idt = np.eye(128, dtype=f32).astype(f8)

    # merged selector: M32[r, r2] = 1 if same batch group
    m32 = np.zeros((32, 32), dtype=f32)
    for r in range(32):
        m32[r, (r // 8) * 8 : (r // 8) * 8 + 8] = 0.0
    for r in range(32):
        for r2 in range(32):
            if r // 8 == r2 // 8:
                m32[r, r2] = 1.0

    in_maps = []
    for c in range(NCORES):
        m = mask[:, c * BC : (c + 1) * BC] + bs[0]
        # msk[b*8+tt, c] = m[tt*512+c, b]
        mskc = m.reshape(NTT, TTILE, BC).transpose(2, 0, 1).reshape(32, TTILE)
        mskl = np.ascontiguousarray(np.concatenate([mskc, m32], axis=1))
        lmc = lmT[:, :, c * BC : (c + 1) * BC]
        lm8c = np.tile(lmc.astype(f8), (1, 1, 32 // BC)).reshape(128, 128)
        bl8c = np.ascontiguousarray(np.concatenate([lm8c, idt], axis=1))
        bl32c = np.ascontiguousarray(
            np.concatenate([wgt.reshape(128, 4 * H), lmc.reshape(128, 16)], axis=1)
        )
        in_maps.append(
            {
                "enc": np.ascontiguousarray(enc_t[c]),
                "win": np.ascontiguousarray(win_full[c]),
                "wkp": wkp,
                "bkp": bkp,
                "bl8": bl8c,
                "bl32": bl32c,
                "bgp": bgp,
                "wsr": wsr,
                "mskl": mskl,
            }
        )
    return in_maps


def _assemble_output(per_core):
    out = np.empty((T, B), dtype=np.float32)
    for c in range(NCORES):
        A = np.asarray(per_core[c], dtype=np.float32)   # (32, 512), row = b*8+tt
        blk = A.reshape(BC, NTT * TTILE).T              # (T, BC)
        out[:, c * BC : (c + 1) * BC] = blk
    return out


def kernel(**inputs):
    from concourse.bass_utils import run_bass_kernel_spmd

    in_maps = _prep_inputs(**inputs)
    nc = _get_program()
    trace = bool(os.environ.get("BASS_TRACE"))
    res = run_bass_kernel_spmd(nc, in_maps, list(range(NCORES)), trace=trace)
    _CACHE["last_results"] = res
    return _assemble_output([r["att"] for r in res.results])


# revision 18
# speedup vs baseline: 1.3962x; 1.3962x over previous
"""Trainium2 Bass kernel for nn_LocalAttention (T=4096, B=32, H=256, L=512, K=32).

Sharding: data-parallel over batch B across 8 cores (BC=4 batch elements/core).

v2.1 dataflow (h on partitions, per core):
  - coeff GEMM col-tiled 4x (m=32 replicated lm at col groups), Wk as fp8
    e3m4 in 4 persistent 1MB tiles, i-outer accumulation so matmuls chase
    the DMA stream.  kernT copies split DVE/ACT, DRAM bounce 4+4 DMAs with
    multi-dim partition APs.
  - conv (K=32) row-tiled via tile_position with win+coef replicated at
    partition bases {0,32,64,96}; enc (fp8) added on the PE by identity
    matmul accumulation into the conv PSUM banks.
  - ACT tanh reads PSUM FD=1024 (2-tt packs), bias = per-partition glob.
  - score: stationary ws (m=32 replicated), tan streams, col-tiled over the
    tt quad into a dedicated 1-bank score pool (2 bufs) so the conv pipeline
    never waits on score/softmax.
  - softmax: scores consolidated into one (32,512) SBUF tile; exp is a
    single ACT call; per-b sums via selector matmuls.
  - DMA: triggers cost ~600ns each and queues are in-order, so loads are
    batched into few instructions and spread over the sync/gpsimd queues.
"""

import os
import sys

import numpy as np

if "/opt/trn_rl_repo" not in sys.path:
    sys.path.insert(0, "/opt/trn_rl_repo")

import ml_dtypes

T, B, H, L, K = 4096, 32, 256, 512, 32
NCORES = 8
BC = B // NCORES          # 4 batches per core
HCH = H // 128            # 2 h-chunks
TTILE = 512
NTT = T // TTILE          # 8 t-tiles per (b, hc)

_CACHE = {}


def _build_program():
    import concourse.bacc as bacc
    import concourse.bass as bass
    import concourse.mybir as mybir
    import concourse.tile as tile
    from contextlib import ExitStack

    dt = mybir.dt
    fp32 = dt.float32
    fp16 = dt.float16
    f8 = dt.float8e3
    ts = bass.ts

    nc = bacc.Bacc(
        "TRN2",
        target_bir_lowering=False,
        debug=False,
        enable_asserts=False,
        num_devices=NCORES,
    )

    enc = nc.dram_tensor("enc", (BC, HCH, 128, T), f8, kind="ExternalInput").ap()
    win = nc.dram_tensor("win", (BC, 128, T), dt.float8e4, kind="ExternalInput").ap()
    wkp = nc.dram_tensor("wkp", (4, 128, 16, 512), f8, kind="ExternalInput").ap()
    bkp = nc.dram_tensor("bkp", (1, K * H), f8, kind="ExternalInput").ap()
    bl8 = nc.dram_tensor("bl8", (128, 256), f8, kind="ExternalInput").ap()
    bl32 = nc.dram_tensor("bl32", (128, 4 * H + 16), fp32, kind="ExternalInput").ap()
    bgp = nc.dram_tensor("bgp", (1, H), fp32, kind="ExternalInput").ap()
    wsr = nc.dram_tensor("wsr", (128, HCH, 32), fp16, kind="ExternalInput").ap()
    mskl = nc.dram_tensor("mskl", (32, TTILE + 32), fp32, kind="ExternalInput").ap()
    att = nc.dram_tensor("att", (32, TTILE), fp32, kind="ExternalOutput").ap()

    TanhF = mybir.ActivationFunctionType.Tanh
    ExpF = mybir.ActivationFunctionType.Exp
    Add = mybir.AluOpType.add

    with tile.TileContext(nc) as tc, ExitStack() as ctx:
        # ---------- pools ----------
        small_pool = ctx.enter_context(tc.tile_pool(name="small", bufs=1))
        enc_pool = ctx.enter_context(tc.tile_pool(name="encp", bufs=1))
        win_pool = ctx.enter_context(tc.tile_pool(name="winp", bufs=1))
        tan_pool = ctx.enter_context(tc.tile_pool(name="tanp", bufs=5))
        scat_pool = ctx.enter_context(tc.tile_pool(name="scatp", bufs=2))
        conv_ps = ctx.enter_context(tc.tile_pool(name="cvps", bufs=3, space="PSUM"))
        score_ps = ctx.enter_context(tc.tile_pool(name="scps", bufs=2, space="PSUM"))
        dram_pool = ctx.enter_context(tc.tile_pool(name="dramp", bufs=1, space="DRAM"))

        # ---------- input loads ----------
        # sync queue (in-order): bl8, wk x4, enc x8  (critical path first)
        bl8_sb = small_pool.tile([128, 256], f8)
        nc.sync.dma_start(bl8_sb[:], bl8)
        lm8_v = bl8_sb[:, 0:128].rearrange("p (i b) -> p i b", i=4)   # (128,4,32)
        id_v = bl8_sb[:, 128:256]

        # head-critical smalls on gpsimd/sync
        bk_sb = small_pool.tile([1, K * H], f8)
        nc.sync.dma_start(bk_sb[:], bkp)
        mskl_sb = small_pool.tile([32, TTILE + 32], fp32)
        nc.gpsimd.dma_start(mskl_sb[:], mskl)
        wsr_sb = small_pool.tile([128, HCH, 32], fp16)
        nc.gpsimd.dma_start(wsr_sb[:], wsr)
        bg_sb = small_pool.tile([1, H], fp32)
        nc.gpsimd.dma_start(bg_sb[:], bgp)

        # wk: 8 half-chunks spread over the 3 trigger queues
        wk_tiles = []
        wk_eng = {(0, 0): nc.sync, (0, 1): nc.sync, (1, 0): nc.sync,
                  (1, 1): nc.scalar, (2, 0): nc.scalar, (2, 1): nc.scalar,
                  (3, 0): nc.gpsimd, (3, 1): nc.gpsimd}
        for i in range(4):
            wk_sb = small_pool.tile([128, 16, TTILE], f8, tag=f"wk{i}")
            for h in range(2):
                wk_eng[(i, h)].dma_start(
                    wk_sb[:, 8 * h : 8 * h + 8, :], wkp[i, :, 8 * h : 8 * h + 8, :]
                )
            wk_tiles.append(wk_sb)

        # win: per-b tiles, 4x partition-replicated on host (fp8, x4096)
        win_tiles = []
        for b in range(BC):
            w_sb = win_pool.tile([128, T], dt.float8e4, tag=f"win{b}")
            nc.gpsimd.dma_start(w_sb[:], win[b])
            win_tiles.append(w_sb)

        bl32_sb = small_pool.tile([128, 4 * H + 16], fp32)
        nc.scalar.dma_start(bl32_sb[:], bl32)
        wgt_v = bl32_sb[:, 0 : 4 * H].rearrange("p (i h) -> p i h", i=4)
        lmf_v = bl32_sb[:, 4 * H : 4 * H + 16].rearrange("p (i b) -> p i b", i=4)

        enc_tiles = []
        enc_eng = {0: nc.sync, 1: nc.scalar, 2: nc.gpsimd}
        for b in range(BC):
            row = []
            for hc in range(HCH):
                e_sb = enc_pool.tile([128, T], f8, tag=f"enc{b}_{hc}")
                enc_eng[(2 * b + hc) % 3].dma_start(e_sb[:], enc[b, hc])
                row.append(e_sb)
            enc_tiles.append(row)

        one8_sb = small_pool.tile([1, 32], f8)
        nc.vector.memset(one8_sb[:], 1.0)
        one_f = small_pool.tile([1, BC], fp32)
        nc.vector.memset(one_f[:], 1.0)

        # ---------- coeff GEMM: col-tiled x4 over jj, i-outer ----------
        # cps[32jj+b, jg*512+c] = (lm @ WkP + bk) for kh col n=(jg*4+jj)*512+c
        kernT = small_pool.tile([128, 4, TTILE], fp16)
        cpsA = conv_ps.tile([128, 1024], fp32, tag="cps", name="coeffA")
        cpsB = conv_ps.tile([128, 1024], fp32, tag="cps", name="coeffB")
        cps_slice = lambda jg: (cpsA if jg < 2 else cpsB)[:, ts(jg % 2, TTILE)]
        bk_v = bk_sb[:].rearrange("p (j n) -> p j n", j=16)
        for i in range(4):
            for jg in range(4):
                for jj in range(4):
                    nc.tensor.matmul(
                        cps_slice(jg)[32 * jj : 32 * jj + 32, :],
                        lm8_v[:, i, :],
                        wk_tiles[i][:, jg * 4 + jj, :],
                        start=(i == 0),
                        stop=False,
                        skip_group_check=True,
                        tile_position=(0, 32 * jj),
                    )
        for jg in range(4):
            for jj in range(4):
                nc.tensor.matmul(
                    cps_slice(jg)[32 * jj : 32 * jj + 32, :],
                    one8_sb[:],
                    bk_v[:, jg * 4 + jj, :],
                    start=False,
                    stop=True,
                    skip_group_check=True,
                    tile_position=(0, 32 * jj),
                )
            if jg % 2 == 0:
                nc.vector.tensor_scalar_mul(kernT[:, jg, :], cps_slice(jg), 1.0 / 4096.0)
            else:
                nc.scalar.mul(kernT[:, jg, :], cps_slice(jg), 1.0 / 4096.0)

        # bounce through DRAM: scr[b, k, hcc, h]; k = 8jg + 2jj + chi
        scr = dram_pool.tile([BC, K, HCH, 128], fp16)
        scr_v = scr[:].rearrange("b (jg jjj chi) c h -> jjj b jg chi c h", jg=4, jjj=4)
        for jj in range(4):
            src = kernT[32 * jj : 32 * jj + BC, :, :].rearrange(
                "b jg (chi c h) -> b jg chi c h", chi=2, c=HCH
            )
            nc.sync.dma_start(scr_v[jj], src)
        # gather back, replicated x4 at partition bases {0,32,64,96}
        coefr = small_pool.tile([128, BC, HCH, 128], fp16)
        scr_k = scr[:].rearrange("b k c h -> k b c h")
        for rep in range(4):
            nc.sync.dma_start(coefr[32 * rep : 32 * rep + 32, :, :, :], scr_k)

        # ---------- glob: glob[h, b] = lm @ Wg.T + bg ----------
        glob_sb = small_pool.tile([128, HCH, BC], fp32)
        gps = conv_ps.tile([128, 1024], fp32, tag="cps", name="globps")
        for hc in range(HCH):
            gout = gps[:, 512 * hc : 512 * hc + BC]
            for i in range(4):
                nc.tensor.matmul(
                    gout,
                    wgt_v[:, i, ts(hc, 128)],
                    lmf_v[:, i, :],
                    start=(i == 0),
                    stop=False,
                )
            nc.tensor.matmul(gout, bg_sb[:, ts(hc, 128)], one_f[:], start=False, stop=True)
            nc.vector.tensor_copy(glob_sb[:, hc, :], gout)

        # ---------- main stream ----------
        scoreall = small_pool.tile([32, TTILE], fp32)

        def emit_score(b, q, tans):
            # score: stationary replicated ws (m=32), col-tiled over quad
            stile = score_ps.tile([128, TTILE], fp32, tag="sc")
            for hc in range(HCH):
                for i in range(4):
                    nc.tensor.matmul(
                        stile[32 * i : 32 * i + 32, :],
                        wsr_sb[:, hc, :],
                        tans[hc][:, ts(i, TTILE)],
                        start=(hc == 0),
                        stop=(hc == HCH - 1),
                        skip_group_check=True,
                        tile_position=(0, 32 * i),
                    )
            scat = scat_pool.tile([128, TTILE], fp32, tag="scat")
            nc.vector.tensor_copy(scat[:], stile[:])
            scat_v = scat[:].rearrange("(j r) c -> j r c", j=4)
            r0 = b * 8 + q * 4
            nc.gpsimd.dma_start(scoreall[r0 : r0 + 4, :], scat_v[:, 0, :])

        prev_score = None
        for b in range(BC):
            for q in range(2):
                tans = []
                for hc in range(HCH):
                    tan_sb = tan_pool.tile([128, 2048], fp16, tag="tan")
                    for half in range(2):
                        cpt = conv_ps.tile([128, 1024], fp32, tag="cps", name="convps")
                        for j in range(2):
                            i = 2 * half + j
                            tt = q * 4 + i
                            nc.tensor.matmul(
                                cpt[:, ts(j, TTILE)],
                                coefr[32 * i : 32 * i + 32, b, hc, :],
                                win_tiles[b][32 * i : 32 * i + 32, ts(tt, TTILE)],
                                start=True,
                                stop=False,
                                skip_group_check=True,
                                tile_position=(32 * i, 0),
                            )
                        for j in range(2):
                            tt = q * 4 + 2 * half + j
                            nc.tensor.matmul(
                                cpt[:, ts(j, TTILE)],
                                id_v,
                                enc_tiles[b][hc][:, ts(tt, TTILE)],
                                start=False,
                                stop=True,
                                skip_group_check=True,
                            )
                        nc.scalar.activation(
                            tan_sb[:, ts(half, 1024)],
                            cpt[:],
                            TanhF,
                            bias=glob_sb[:, hc, b : b + 1],
                            scale=1.0,
                        )
                    tans.append(tan_sb)
                if prev_score is not None:
                    emit_score(*prev_score)
                prev_score = (b, q, tans)
        emit_score(*prev_score)

        # ---------- softmax over T (per batch) ----------
        sc2 = small_pool.tile([32, TTILE], fp32)
        nc.vector.tensor_tensor(sc2[:], scoreall[:], mskl_sb[:, 0:TTILE], Add)
        esb = small_pool.tile([32, TTILE], fp32)
        rsum = small_pool.tile([32, 1], fp32)
        nc.scalar.activation(esb[:], sc2[:], ExpF, bias=0.0, scale=1.0,
                             accum_out=rsum[:])
        spt = score_ps.tile([128, TTILE], fp32, tag="sc", name="smps")
        nc.tensor.matmul(
            spt[0:32, 0:1], mskl_sb[:, TTILE : TTILE + 32], rsum[:], start=True, stop=True
        )
        rec = small_pool.tile([32, 1], fp32)
        nc.vector.reciprocal(rec[:], spt[0:32, 0:1])
        attall = small_pool.tile([32, TTILE], fp32)
        nc.vector.tensor_scalar_mul(attall[:], esb[:], rec[:])
        nc.sync.dma_start(att, attall[:])

    nc.compile()
    return nc


def _get_program():
    if "nc" not in _CACHE:
        _CACHE["nc"] = _build_program()
    return _CACHE["nc"]


def _prep_inputs(encoded_contribution, mask, lm_state, prev_att_weights,
                 Wk, bk, Wg, bg, Ws, bs):
    """Host-side shard + layout prep. Returns list of per-core input dicts."""
    f32 = np.float32
    f8 = ml_dtypes.float8_e3m4

    enc = np.asarray(encoded_contribution, dtype=f32)
    mask = np.asarray(mask, dtype=f32)
    lm = np.asarray(lm_state, dtype=f32)
    prev = np.asarray(prev_att_weights, dtype=f32)
    Wk = np.asarray(Wk, dtype=f32)
    bk = np.asarray(bk, dtype=f32)
    Wg = np.asarray(Wg, dtype=f32)
    bg = np.asarray(bg, dtype=f32)
    Ws = np.asarray(Ws, dtype=f32)
    bs = np.asarray(bs, dtype=f32)

    # enc: (T, B, H) -> (B, H, T) -> (NCORES, BC, HCH, 128, T) fp8 e3m4
    enc_t = np.ascontiguousarray(enc.transpose(1, 2, 0)).astype(f8).reshape(
        NCORES, BC, HCH, 128, T
    )

    # toeplitz windows: win[b, k, t] = prev_pad[b, k + t]
    prev_pad = np.zeros((B, T + K - 1), dtype=f32)
    prev_pad[:, K - 1 :] = prev.T
    win_full = np.lib.stride_tricks.sliding_window_view(prev_pad, T, axis=1)
    win_full = (win_full * 4096.0).astype(ml_dtypes.float8_e4m3).reshape(NCORES, BC, K, T)
    # replicate each (K, T) block 4x along partitions -> (NCORES, BC, 128, T)
    win_full = np.ascontiguousarray(np.tile(win_full, (1, 1, 4, 1)))

    # WkP[l, k*256+h] = Wk[h*32+k, l]; dram layout (4 i, 128 p, 16 j, 512 n)
    wkp = (
        Wk.reshape(H, K, L)
        .transpose(2, 1, 0)          # (L, K, H)
        .reshape(L, K * H)
        .astype(f8)
        .reshape(4, 128, 16, 512)
    )
    wkp = np.ascontiguousarray(wkp)

    # bk permuted to [k*256+h]
    bkp = np.ascontiguousarray(bk.reshape(H, K).T.reshape(1, K * H)).astype(f8)

    # lmT chunks: (128, 4, B); per core slice then tile to 32 cols
    lmT = np.ascontiguousarray(lm.T.reshape(4, 128, B).transpose(1, 0, 2))

    # WgT chunks: (128, 4, H) flattened into bl32
    wgt = np.ascontiguousarray(Wg.T.reshape(4, 128, H).transpose(1, 0, 2))
    bgp = np.ascontiguousarray(bg.reshape(1, H))

    # ws replicated: (128, HCH, 32)
    wsv = Ws[0].reshape(HCH, 128).T.astype(np.float16)      # (128, HCH)
    wsr = np.ascontiguousarray(np.repeat(wsv[:, :, None], 32, axis=2))

    idt = np.eye(128, dtype=f32).astype(f8)

    # merged selector: M32[r, r2] = 1 if same batch group
    m32 = np.zeros((32, 32), dtype=f32)
    for r in range(32):
        m32[r, (r // 8) * 8 : (r // 8) * 8 + 8] = 0.0
    for r in range(32):
        for r2 in range(32):
            if r // 8 == r2 // 8:
                m32[r, r2] = 1.0

    in_maps = []
    for c in range(NCORES):
        m = mask[:, c * BC : (c + 1) * BC] + bs[0]
        # msk[b*8+tt, c] = m[tt*512+c, b]
        mskc = m.reshape(NTT, TTILE, BC).transpose(2, 0, 1).reshape(32, TTILE)
        mskl = np.ascontiguousarray(np.concatenate([mskc, m32], axis=1))
        lmc = lmT[:, :, c * BC : (c + 1) * BC]
        lm8c = np.tile(lmc.astype(f8), (1, 1, 32 // BC)).reshape(128, 128)
        bl8c = np.ascontiguousarray(np.concatenate([lm8c, idt], axis=1))
        bl32c = np.ascontiguousarray(
            np.concatenate([wgt.reshape(128, 4 * H), lmc.reshape(128, 16)], axis=1)
        )
        in_maps.append(
            {
                "enc": np.ascontiguousarray(enc_t[c]),
                "win": np.ascontiguousarray(win_full[c]),
                "wkp": wkp,
                "bkp": bkp,
                "bl8": bl8c,
                "bl32": bl32c,
                "bgp": bgp,
                "wsr": wsr,
                "mskl": mskl,
            }
        )
    return in_maps


def _assemble_output(per_core):
    out = np.empty((T, B), dtype=np.float32)
    for c in range(NCORES):
        A = np.asarray(per_core[c], dtype=np.float32)   # (32, 512), row = b*8+tt
        blk = A.reshape(BC, NTT * TTILE).T              # (T, BC)
        out[:, c * BC : (c + 1) * BC] = blk
    return out


def kernel(**inputs):
    from concourse.bass_utils import run_bass_kernel_spmd

    in_maps = _prep_inputs(**inputs)
    nc = _get_program()
    trace = bool(os.environ.get("BASS_TRACE"))
    res = run_bass_kernel_spmd(nc, in_maps, list(range(NCORES)), trace=trace)
    _CACHE["last_results"] = res
    return _assemble_output([r["att"] for r in res.results])
